# revision 6
# baseline (speedup 1.0000x reference)
"""Causal self-attention Bass/Tile kernel for 8 TRN2 NeuronCores.

Sharding: core c handles batch b = c//2 and heads h in [8*(c%2), 8*(c%2)+8).
Each core computes a partial projection output (its 512 channels' worth of the
contraction); the host sums the two partials per batch.

Layouts on device (per core):
  x:      [T, C] DRAM  -> x.T tiles [c, t] in SBUF via PE transpose
  q,k:    [j, t] (d-on-partition) from QKV matmul  (lhsT = W.T, rhs = x.T)
  v:      [t, j] (t-on-partition) from QKV matmul  (lhsT = x.T, rhs = Wv.T)
  scores: S.T [tk, tq] = k-tile.T @ q-chunk, 2 heads row-tiled (K=64 each)
  P = exp(S/8) via ACT (psum -> bf16 sbuf), causal mask via affine_select
  AV:     y.T [d|l, tq] += (v|1).T @ P per tk tile  (bf16, fp32 accum)
  proj:   out [t, c_out] = y.T-tiles.T @ Wproj.T   (fp32r)
"""

import sys

if "/opt/trn_rl_repo" not in sys.path:
    sys.path.insert(0, "/opt/trn_rl_repo")

import numpy as np

import concourse.bass as bass
import concourse.mybir as mybir
import concourse.tile as tile
from concourse import bacc, bass_utils
from concourse.masks import make_identity

F32 = mybir.dt.float32
F32R = mybir.dt.float32r
BF16 = mybir.dt.bfloat16

B, T, C = 4, 2048, 1024
H = 16
D = 64
JL = 512          # local channels per q/k/v slice (8 heads * 64)
P = 128
NCHUNK = T // 512  # 4 tq/t chunks of 512
NPAIR = 4          # head pairs per core (8 local heads)


def build_nc():
    nc = bacc.Bacc("TRN2", target_bir_lowering=False, debug=False)
    x = nc.dram_tensor("x", [T, C], F32, kind="ExternalInput").ap()
    wqkv = nc.dram_tensor("wqkv", [3 * JL, C], F32, kind="ExternalInput").ap()
    wproj = nc.dram_tensor("wproj", [C, JL], F32, kind="ExternalInput").ap()
    out = nc.dram_tensor("out", [T, C], F32, kind="ExternalOutput").ap()

    CT = C // P       # 8 c-tiles
    Exp = mybir.ActivationFunctionType.Exp

    with tile.TileContext(nc) as tc:
        with (
            tc.tile_pool(name="singles", bufs=1) as singles,
            tc.tile_pool(name="xnat", bufs=2) as xnat_pool,
            tc.tile_pool(name="xT", bufs=1) as xT_pool,
            tc.tile_pool(name="qsb", bufs=2) as qsb_pool,
            tc.tile_pool(name="pt", bufs=4) as pt_pool,
            tc.tile_pool(name="yT", bufs=2) as yT_pool,
            tc.tile_pool(name="ob", bufs=2) as ob_pool,
            tc.tile_pool(name="small", bufs=4) as small_pool,
            tc.tile_pool(name="ps_sc", bufs=4, space="PSUM") as ps_sc,
            tc.tile_pool(name="ps_av", bufs=2, space="PSUM") as ps_av,
            tc.tile_pool(name="ps_mm", bufs=2, space="PSUM") as ps_mm,
        ):
            identity = singles.tile([P, P], F32)
            make_identity(nc, identity)
            ones64f = singles.tile([1, D], F32)
            nc.vector.memset(ones64f, 1.0)
            ones64 = singles.tile([1, D], F32R)
            nc.vector.tensor_copy(ones64, ones64f)

            # persistent tensors
            wqkT = singles.tile([P, 8, CT, P], F32R)    # [c, jt, cc, j] 32KB/part
            wvT = singles.tile([P, CT, JL], F32R)       # [c, cc, j]     16KB/part
            wprojT = singles.tile([P, 4, C], F32R)      # [j, g, c_out]  16KB/part
            k_sb = singles.tile([P, 4, T], F32R)        # [d2, hp, tk]   32KB/part
            v_sb = singles.tile([P, T // P, 8, D + 1], BF16)  # [t, tkt, h, d|1]
            nc.vector.memset(v_sb[:, :, :, D], 1.0)

            # ---- stage 0: transpose weights ----
            for jt in range(12):
                wn = xnat_pool.tile([P, C], F32, tag="xn")
                nc.sync.dma_start(out=wn, in_=wqkv[jt * P:(jt + 1) * P, :])
                for cc in range(CT):
                    ptile = ps_mm.tile([P, P], F32, tag="mm")
                    nc.tensor.transpose(ptile, wn[:, cc * P:(cc + 1) * P], identity)
                    if jt < 8:
                        nc.vector.tensor_copy(out=wqkT[:, jt, cc, :], in_=ptile)
                    else:
                        nc.vector.tensor_copy(
                            out=wvT[:, cc, (jt - 8) * P:(jt - 7) * P], in_=ptile
                        )
            for ct in range(8):
                wp = xnat_pool.tile([P, JL], F32, tag="wp")
                nc.sync.dma_start(out=wp, in_=wproj[ct * P:(ct + 1) * P, :])
                for g in range(4):
                    ptile = ps_mm.tile([P, P], F32, tag="mm")
                    nc.tensor.transpose(ptile, wp[:, g * P:(g + 1) * P], identity)
                    nc.vector.tensor_copy(
                        out=wprojT[:, g, ct * P:(ct + 1) * P], in_=ptile
                    )

            for q in range(NCHUNK):
                t0 = q * 512
                # ---- QKV for t-chunk q ----
                xT = xT_pool.tile([P, CT, 512], F32R)
                for tt in range(4):
                    xn = xnat_pool.tile([P, C], F32, tag="xn")
                    r0 = t0 + tt * P
                    nc.sync.dma_start(out=xn, in_=x[r0:r0 + P, :])
                    for cc in range(CT):
                        ptile = ps_mm.tile([P, P], F32, tag="mm")
                        nc.tensor.transpose(
                            ptile, xn[:, cc * P:(cc + 1) * P], identity
                        )
                        nc.vector.tensor_copy(
                            out=xT[:, cc, tt * P:(tt + 1) * P], in_=ptile
                        )
                # v in [t, j]
                for tt in range(4):
                    pv = ps_mm.tile([P, JL], F32, tag="mm")
                    for cc in range(CT):
                        nc.tensor.matmul(
                            pv,
                            lhsT=xT[:, cc, tt * P:(tt + 1) * P],
                            rhs=wvT[:, cc, :],
                            start=(cc == 0),
                            stop=(cc == CT - 1),
                        )
                    for h in range(8):
                        nc.vector.tensor_copy(
                            out=v_sb[:, q * 4 + tt, h, 0:D],
                            in_=pv[:, h * D:(h + 1) * D],
                        )
                # q, k in [j, t]
                q_sb = qsb_pool.tile([P, 4, 512], F32R)
                for jt in range(8):
                    pq = ps_mm.tile([P, 512], F32, tag="mm")
                    for cc in range(CT):
                        nc.tensor.matmul(
                            pq,
                            lhsT=wqkT[:, jt, cc, :],
                            rhs=xT[:, cc, :],
                            start=(cc == 0),
                            stop=(cc == CT - 1),
                        )
                    if jt < 4:
                        nc.vector.tensor_copy(out=q_sb[:, jt, :], in_=pq)
                    else:
                        nc.vector.tensor_copy(
                            out=k_sb[:, jt - 4, t0:t0 + 512], in_=pq
                        )

                # ---- attention for tq-chunk q, all head pairs ----
                yT = yT_pool.tile([P, 4, 512], F32R)
                ntk = 4 * q + 4
                for hp in range(NPAIR):
                    pav0 = ps_av.tile([D + 1, 512], F32, tag="av")
                    pav1 = ps_av.tile([D + 1, 512], F32, tag="av")
                    pav = [pav0, pav1]
                    for j in range(ntk):
                        for h2 in range(2):
                            ps = ps_sc.tile([P, 512], F32, tag="sc")
                            nc.tensor.matmul(
                                ps,
                                lhsT=k_sb[
                                    h2 * D:(h2 + 1) * D, hp, j * P:(j + 1) * P
                                ],
                                rhs=q_sb[h2 * D:(h2 + 1) * D, hp, :],
                                start=True,
                                stop=True,
                            )
                            pt = pt_pool.tile([P, 512], BF16, tag="pt")
                            nc.scalar.activation(pt, ps, Exp, scale=0.125)
                            if j >= 4 * q:
                                # zero where tq < tk: keep f - p - o >= 0
                                o = j * P - t0
                                nc.gpsimd.affine_select(
                                    out=pt,
                                    in_=pt,
                                    pattern=[[1, 512]],
                                    compare_op=mybir.AluOpType.is_ge,
                                    fill=0.0,
                                    base=-o,
                                    channel_multiplier=-1,
                                )
                            nc.tensor.matmul(
                                pav[h2],
                                lhsT=v_sb[:, j, hp * 2 + h2, :],
                                rhs=pt,
                                start=(j == 0),
                                stop=(j == ntk - 1),
                            )
                    for h2 in range(2):
                        l_sb = small_pool.tile([1, 512], F32R, tag="l")
                        nc.vector.tensor_copy(l_sb, pav[h2][D:D + 1, :])
                        pb = ps_sc.tile([D, 512], F32, tag="sc")
                        nc.tensor.matmul(
                            pb,
                            lhsT=ones64,
                            rhs=l_sb,
                            start=True,
                            stop=True,
                        )
                        pbs = small_pool.tile([D, 512], F32, tag="pbs")
                        nc.vector.reciprocal(pbs, pb)
                        nc.vector.tensor_mul(
                            yT[h2 * D:(h2 + 1) * D, hp, :],
                            pav[h2][0:D, :],
                            pbs,
                        )

                # ---- proj for t-chunk q ----
                for tt in range(4):
                    for ct in range(2):
                        po = ps_mm.tile([P, 512], F32, tag="mm")
                        for g in range(4):
                            nc.tensor.matmul(
                                po,
                                lhsT=yT[:, g, tt * P:(tt + 1) * P],
                                rhs=wprojT[:, g, ct * 512:(ct + 1) * 512],
                                start=(g == 0),
                                stop=(g == 3),
                            )
                        ob = ob_pool.tile([P, 512], F32, tag="ob")
                        nc.vector.tensor_copy(ob, po)
                        nc.sync.dma_start(
                            out=out[
                                t0 + tt * P:t0 + (tt + 1) * P,
                                ct * 512:(ct + 1) * 512,
                            ],
                            in_=ob,
                        )

    nc.compile()
    return nc


_NC = None


def _get_nc():
    global _NC
    if _NC is None:
        _NC = build_nc()
    return _NC


def _shard_inputs(x, w_attn, w_proj):
    in_maps = []
    for c in range(8):
        b, s = c // 2, c % 2
        j0 = s * JL
        wqkv_c = np.concatenate(
            [
                w_attn[j0:j0 + JL],
                w_attn[C + j0:C + j0 + JL],
                w_attn[2 * C + j0:2 * C + j0 + JL],
            ],
            axis=0,
        )
        in_maps.append(
            {
                "x": np.ascontiguousarray(x[b]).astype(np.float32),
                "wqkv": np.ascontiguousarray(wqkv_c).astype(np.float32),
                "wproj": np.ascontiguousarray(w_proj[:, j0:j0 + JL]).astype(
                    np.float32
                ),
            }
        )
    return in_maps


def run(x, w_attn, w_proj, **run_kwargs):
    """Run on 8 cores; returns (out [B,T,C], BassKernelResults)."""
    nc = _get_nc()
    in_maps = _shard_inputs(np.asarray(x), np.asarray(w_attn), np.asarray(w_proj))
    res = bass_utils.run_bass_kernel_spmd(
        nc, in_maps, core_ids=list(range(8)), **run_kwargs
    )
    out = np.empty((B, T, C), dtype=np.float32)
    for b in range(B):
        out[b] = res.results[2 * b]["out"] + res.results[2 * b + 1]["out"]
    return out, res


def kernel(x, w_attn, w_proj):
    return run(x, w_attn, w_proj)[0]


# revision 9
# speedup vs baseline: 1.1263x; 1.1263x over previous
"""Causal self-attention Bass/Tile kernel for 8 TRN2 NeuronCores.

Sharding: core c handles batch b = c//2 and heads h in [8*(c%2), 8*(c%2)+8).
Each core computes a partial projection output (its 512 channels' worth of the
contraction); the host sums the two partials per batch.

Layouts on device (per core):
  x:      [T, C] DRAM  -> x.T tiles [c, t] in SBUF via PE transpose
  q,k:    [j, t] (d-on-partition) from QKV matmul  (lhsT = W.T, rhs = x.T)
  v:      [t, j] (t-on-partition) from QKV matmul  (lhsT = x.T, rhs = Wv.T)
  scores: S.T [tk, tq] = k-tile.T @ q-chunk, 2 heads row-tiled (K=64 each)
  P = exp(S/8) via ACT (psum -> bf16 sbuf), causal mask via affine_select
  AV:     y.T [d|l, tq] += (v|1).T @ P per tk tile  (bf16, fp32 accum)
  proj:   out [t, c_out] = y.T-tiles.T @ Wproj.T   (fp32r)
"""

import sys

if "/opt/trn_rl_repo" not in sys.path:
    sys.path.insert(0, "/opt/trn_rl_repo")

import numpy as np

import concourse.bass as bass
import concourse.mybir as mybir
import concourse.tile as tile
from concourse import bacc, bass_utils
from concourse.masks import make_identity

F32 = mybir.dt.float32
F32R = mybir.dt.float32r
BF16 = mybir.dt.bfloat16

B, T, C = 4, 2048, 1024
H = 16
D = 64
JL = 512          # local channels per q/k/v slice (8 heads * 64)
P = 128
NCHUNK = T // 512  # 4 tq/t chunks of 512
NPAIR = 4          # head pairs per core (8 local heads)


def build_nc():
    nc = bacc.Bacc("TRN2", target_bir_lowering=False, debug=False)
    x = nc.dram_tensor("x", [T, C], F32, kind="ExternalInput").ap()
    wqkv = nc.dram_tensor("wqkv", [3 * JL, C], F32, kind="ExternalInput").ap()
    wproj = nc.dram_tensor("wproj", [C, JL], F32, kind="ExternalInput").ap()
    out = nc.dram_tensor("out", [T, C], F32, kind="ExternalOutput").ap()

    CT = C // P       # 8 c-tiles
    Exp = mybir.ActivationFunctionType.Exp

    with tile.TileContext(nc) as tc:
        with (
            tc.tile_pool(name="singles", bufs=1) as singles,
            tc.tile_pool(name="xnat", bufs=2) as xnat_pool,
            tc.tile_pool(name="xT", bufs=1) as xT_pool,
            tc.tile_pool(name="qsb", bufs=2) as qsb_pool,
            tc.tile_pool(name="pt", bufs=4) as pt_pool,
            tc.tile_pool(name="yT", bufs=2) as yT_pool,
            tc.tile_pool(name="ob", bufs=2) as ob_pool,
            tc.tile_pool(name="small", bufs=2) as small_pool,
            tc.tile_pool(name="ps_sc", bufs=2, space="PSUM") as ps_sc,
            tc.tile_pool(name="ps_av", bufs=2, space="PSUM") as ps_av,
            tc.tile_pool(name="ps_mm", bufs=2, space="PSUM") as ps_mm,
        ):
            identity = singles.tile([P, P], F32)
            make_identity(nc, identity)
            ones64f = singles.tile([1, D], F32)
            nc.vector.memset(ones64f, 1.0)
            ones64 = singles.tile([1, D], F32R)
            nc.vector.tensor_copy(ones64, ones64f)

            # persistent tensors
            wqkT = singles.tile([P, 8, CT, P], F32R)    # [c, jt, cc, j] 32KB/part
            wvT = singles.tile([P, CT, JL], F32R)       # [c, cc, j]     16KB/part
            wprojT = singles.tile([P, 4, C], F32R)      # [j, g, c_out]  16KB/part
            k_sb = singles.tile([P, 4, T], F32R)        # [d2, hp, tk]   32KB/part
            v_sb = singles.tile([P, T // P, 8, D + 1], BF16)  # [t, tkt, h, d|1]
            nc.vector.memset(v_sb[:, :, :, D], 1.0)

            # ---- stage 0: transpose weights ----
            for jt in range(12):
                wn = xnat_pool.tile([P, C], F32, tag="xn")
                nc.sync.dma_start(out=wn, in_=wqkv[jt * P:(jt + 1) * P, :])
                for cc in range(CT):
                    ptile = ps_sc.tile([P, P], F32, tag="sc")
                    nc.tensor.transpose(ptile, wn[:, cc * P:(cc + 1) * P], identity)
                    if jt < 8:
                        nc.vector.tensor_copy(out=wqkT[:, jt, cc, :], in_=ptile)
                    else:
                        nc.vector.tensor_copy(
                            out=wvT[:, cc, (jt - 8) * P:(jt - 7) * P], in_=ptile
                        )
            for ct in range(8):
                wp = xnat_pool.tile([P, JL], F32, tag="wp")
                nc.sync.dma_start(out=wp, in_=wproj[ct * P:(ct + 1) * P, :])
                for g in range(4):
                    ptile = ps_sc.tile([P, P], F32, tag="sc")
                    nc.tensor.transpose(ptile, wp[:, g * P:(g + 1) * P], identity)
                    nc.vector.tensor_copy(
                        out=wprojT[:, g, ct * P:(ct + 1) * P], in_=ptile
                    )

            for q in range(NCHUNK):
                t0 = q * 512
                # ---- QKV for t-chunk q ----
                xT = xT_pool.tile([P, CT, 512], F32R)
                for tt in range(4):
                    xn = xnat_pool.tile([P, C], F32, tag="xn")
                    r0 = t0 + tt * P
                    nc.sync.dma_start(out=xn, in_=x[r0:r0 + P, :])
                    for cc in range(CT):
                        ptile = ps_sc.tile([P, P], F32, tag="sc")
                        nc.tensor.transpose(
                            ptile, xn[:, cc * P:(cc + 1) * P], identity
                        )
                        nc.vector.tensor_copy(
                            out=xT[:, cc, tt * P:(tt + 1) * P], in_=ptile
                        )
                # v in [t, j]
                for tt in range(4):
                    pv = ps_mm.tile([P, JL], F32, tag="mm")
                    for cc in range(CT):
                        nc.tensor.matmul(
                            pv,
                            lhsT=xT[:, cc, tt * P:(tt + 1) * P],
                            rhs=wvT[:, cc, :],
                            start=(cc == 0),
                            stop=(cc == CT - 1),
                        )
                    for h in range(8):
                        nc.vector.tensor_copy(
                            out=v_sb[:, q * 4 + tt, h, 0:D],
                            in_=pv[:, h * D:(h + 1) * D],
                        )
                # q, k in [j, t]
                q_sb = qsb_pool.tile([P, 4, 512], F32R)
                for jt in range(8):
                    pq = ps_mm.tile([P, 512], F32, tag="mm")
                    for cc in range(CT):
                        nc.tensor.matmul(
                            pq,
                            lhsT=wqkT[:, jt, cc, :],
                            rhs=xT[:, cc, :],
                            start=(cc == 0),
                            stop=(cc == CT - 1),
                        )
                    if jt < 4:
                        nc.vector.tensor_copy(out=q_sb[:, jt, :], in_=pq)
                    else:
                        nc.vector.tensor_copy(
                            out=k_sb[:, jt - 4, t0:t0 + 512], in_=pq
                        )

                # ---- attention for tq-chunk q, all head pairs ----
                yT = yT_pool.tile([P, 4, 512], F32R)
                ntk = 4 * q + 4
                for hp in range(NPAIR):
                    pav0 = ps_av.tile([D + 1, 512], F32, tag="av")
                    pav1 = ps_av.tile([D + 1, 512], F32, tag="av")
                    pav = [pav0, pav1]
                    for j in range(ntk):
                        ps = ps_sc.tile([P, 2, 512], F32, tag="sc")
                        for h2 in range(2):
                            nc.tensor.matmul(
                                ps[:, h2, :],
                                lhsT=k_sb[
                                    h2 * D:(h2 + 1) * D, hp, j * P:(j + 1) * P
                                ],
                                rhs=q_sb[h2 * D:(h2 + 1) * D, hp, :],
                                start=True,
                                stop=True,
                            )
                        pt = pt_pool.tile([P, 2, 512], BF16, tag="pt")
                        nc.scalar.activation(pt, ps, Exp, scale=0.125)
                        if j >= 4 * q:
                            # zero where tq < tk: keep f - p - o >= 0 (per half)
                            o = j * P - t0
                            nc.gpsimd.affine_select(
                                out=pt,
                                in_=pt,
                                pattern=[[0, 2], [1, 512]],
                                compare_op=mybir.AluOpType.is_ge,
                                fill=0.0,
                                base=-o,
                                channel_multiplier=-1,
                            )
                        for h2 in range(2):
                            nc.tensor.matmul(
                                pav[h2],
                                lhsT=v_sb[:, j, hp * 2 + h2, :],
                                rhs=pt[:, h2, :],
                                start=(j == 0),
                                stop=(j == ntk - 1),
                            )
                    for h2 in range(2):
                        l_f = small_pool.tile([1, 512], F32, tag="lf")
                        nc.vector.reciprocal(l_f, pav[h2][D:D + 1, :])
                        l_sb = small_pool.tile([1, 512], F32R, tag="l")
                        nc.vector.tensor_copy(l_sb, l_f)
                        pb = ps_sc.tile([D, 512], F32, tag="sc")
                        nc.tensor.matmul(
                            pb,
                            lhsT=ones64,
                            rhs=l_sb,
                            start=True,
                            stop=True,
                        )
                        pbs = small_pool.tile([D, 512], F32, tag="pbs")
                        nc.vector.tensor_copy(pbs, pb)
                        nc.vector.tensor_mul(
                            yT[h2 * D:(h2 + 1) * D, hp, :],
                            pav[h2][0:D, :],
                            pbs,
                        )

                # ---- proj for t-chunk q ----
                for tt in range(4):
                    for ct in range(2):
                        po = ps_mm.tile([P, 512], F32, tag="mm")
                        for g in range(4):
                            nc.tensor.matmul(
                                po,
                                lhsT=yT[:, g, tt * P:(tt + 1) * P],
                                rhs=wprojT[:, g, ct * 512:(ct + 1) * 512],
                                start=(g == 0),
                                stop=(g == 3),
                            )
                        ob = ob_pool.tile([P, 512], F32, tag="ob")
                        nc.vector.tensor_copy(ob, po)
                        nc.sync.dma_start(
                            out=out[
                                t0 + tt * P:t0 + (tt + 1) * P,
                                ct * 512:(ct + 1) * 512,
                            ],
                            in_=ob,
                        )

    nc.compile()
    return nc


_NC = None


def _get_nc():
    global _NC
    if _NC is None:
        _NC = build_nc()
    return _NC


def _shard_inputs(x, w_attn, w_proj):
    in_maps = []
    for c in range(8):
        b, s = c // 2, c % 2
        j0 = s * JL
        wqkv_c = np.concatenate(
            [
                w_attn[j0:j0 + JL],
                w_attn[C + j0:C + j0 + JL],
                w_attn[2 * C + j0:2 * C + j0 + JL],
            ],
            axis=0,
        )
        in_maps.append(
            {
                "x": np.ascontiguousarray(x[b]).astype(np.float32),
                "wqkv": np.ascontiguousarray(wqkv_c).astype(np.float32),
                "wproj": np.ascontiguousarray(w_proj[:, j0:j0 + JL]).astype(
                    np.float32
                ),
            }
        )
    return in_maps


def run(x, w_attn, w_proj, **run_kwargs):
    """Run on 8 cores; returns (out [B,T,C], BassKernelResults)."""
    nc = _get_nc()
    in_maps = _shard_inputs(np.asarray(x), np.asarray(w_attn), np.asarray(w_proj))
    res = bass_utils.run_bass_kernel_spmd(
        nc, in_maps, core_ids=list(range(8)), **run_kwargs
    )
    out = np.empty((B, T, C), dtype=np.float32)
    for b in range(B):
        out[b] = res.results[2 * b]["out"] + res.results[2 * b + 1]["out"]
    return out, res


def kernel(x, w_attn, w_proj):
    return run(x, w_attn, w_proj)[0]
